# revision 5
# baseline (speedup 1.0000x reference)
"""Trainium2 Bass kernel for nn_LASCC (sparse patch-correlation attention + top-k).

Math (per batch element b):
  x_hat = L2-normalize(x, dim=channels)
  z_p[c, n] = x_hat at the two in-patch diagonal pixels (p=0: (0,0), p=1: (1,1))
  C_p = z_p^T z_p                  (1024x1024 normalized correlation, symmetric)
  C_2 = (C_0 + C_1)/2              (avg map)
  s_q = alpha * mask * C_q         (mask symmetric -> s symmetric)
  A_q = softmax_row(s) * softmax_col(s) = E^2 / (r[n] r[m]),  E=exp(s), r=rowsum(E)
  out pixel (row,col) with patch n, map q: top-3 over m of A_q[n, m]

Device kernel per core handles 2 batch elements (pure data parallel over b).
Top-3 of A[n,:] = (top-3 of F[n,:]*u[m]) * u[n]  with F = E^2 = exp(2*s), u = 1/r.
Top-8 per row via the DVE max8 instruction; first 3 taken.
"""
import numpy as np

import concourse.bass as bass
import concourse.mybir as mybir
from concourse import bacc
from concourse.tile import TileContext
from concourse.bass_utils import run_bass_kernel_spmd

F32 = mybir.dt.float32
F32R = mybir.dt.float32r
BF16 = mybir.dt.bfloat16
AF = mybir.ActivationFunctionType
ALU = mybir.AluOpType

B_FULL = 16
N_CORES = 8
B_LOC = B_FULL // N_CORES  # 2
C = 128
H = W = 64
NPH = 32
NP = 1024  # patches
PS = 2
TOPK = 3
NCHUNK = NP // 128  # 8

LAST_EXEC_NS = None


def _build_mask() -> np.ndarray:
    """(1 - gaussian) self-suppression mask, [NP, NP] float32 (matches reference)."""
    rat_s = np.float32(0.05)
    sr = np.float32(NPH) * rat_s
    ind_r = np.arange(NPH, dtype=np.float32).reshape(1, NPH, 1)
    ind_c = np.arange(NPH, dtype=np.float32).reshape(1, 1, NPH)
    cent = np.arange(NPH, dtype=np.float32)
    cent_r = np.repeat(cent, NPH).reshape(NP, 1, 1)
    cent_c = np.tile(cent, NPH).reshape(NP, 1, 1)
    g = np.exp(-((ind_r - cent_r) ** 2) / (2.0 * sr * sr)) * np.exp(
        -((ind_c - cent_c) ** 2) / (2.0 * sr * sr)
    )
    return (1.0 - g).reshape(NP, NP).astype(np.float32)


def build_nc():
    nc = bacc.Bacc(trn_type="TRN2")

    x_d = nc.dram_tensor("x", [B_LOC, C, H * W], F32, kind="ExternalInput")
    mask_d = nc.dram_tensor("mask", [NP, NP], F32, kind="ExternalInput")
    alpha_d = nc.dram_tensor("alpha", [1, 1], F32, kind="ExternalInput")
    out_d = nc.dram_tensor("out", [B_LOC, 3, NP, TOPK], F32, kind="ExternalOutput")

    with TileContext(nc) as tc:
        with tc.tile_pool(name="const", bufs=1) as cpool, \
             tc.tile_pool(name="big", bufs=1) as big, \
             tc.tile_pool(name="slab", bufs=2) as slabp, \
             tc.tile_pool(name="work", bufs=2) as work, \
             tc.tile_pool(name="small", bufs=2) as small, \
             tc.tile_pool(name="ps", bufs=2, space="PSUM") as ps, \
             tc.tile_pool(name="dsc", bufs=2, space="DRAM") as dsc:

            # ---- constants
            mask_sb = cpool.tile([128, NCHUNK, NP], F32)  # mask[128i+p, m] at [p,i,m]
            nc.sync.dma_start(
                mask_sb, mask_d[:, :].rearrange("(i p) m -> p i m", p=128)
            )
            ones_r = cpool.tile([1, 128], F32)  # K=1 bcast matmul lhsT
            nc.vector.memset(ones_r, 1.0)
            ones_k = cpool.tile([128, 1], F32)  # column-sum matmul lhsT
            nc.vector.memset(ones_k, 1.0)
            al_sb = cpool.tile([1, 1], F32)
            nc.sync.dma_start(al_sb, alpha_d[:, :])
            av = cpool.tile([128, 1], F32)  # alpha
            nc.gpsimd.partition_broadcast(av, al_sb)
            av_h = cpool.tile([128, 1], F32)  # alpha/2
            nc.vector.tensor_scalar_mul(av_h, av, 0.5)
            av_d = cpool.tile([128, 1], F32)  # 2*alpha
            nc.vector.tensor_scalar_mul(av_d, av, 2.0)
            scale_E = [av, av, av_h]
            scale_F = [av_d, av_d, av]

            # ---- phase N: norms + normalized z' for both batches
            # z view: x[b] as [c, (i r j s)] with flat = 128i + 64r + 2j + s
            zp = []  # zp[b][p] normalized [128, NP]
            for b in range(B_LOC):
                xs = big.tile([128, H * W], F32, name=f"xs{b}", tag=f"xs{b}", bufs=1)
                nc.sync.dma_start(xs, x_d[b])
                xr = xs.rearrange("c (i r j s) -> c r s i j", r=PS, s=PS, j=NPH)
                zb = []
                for p in range(PS):
                    zv = xr[:, p, p]  # [128, 32, 32] strided view of diag pixels
                    zsq = work.tile([128, NP], F32, name="zsq", tag="zsq")
                    nc.scalar.activation(
                        zsq.rearrange("c (a b) -> c a b", a=NPH), zv, AF.Square)
                    nrm = ps.tile([1, NP], F32, name="nrm", tag="bigbc")
                    for h in range(2):
                        nc.tensor.matmul(nrm[:, 512 * h:512 * (h + 1)], ones_k,
                                         zsq[:, 512 * h:512 * (h + 1)],
                                         start=True, stop=True)
                    nrm_sb = small.tile([1, NP], F32, name="nrm_sb", tag="nrm_sb")
                    nc.scalar.copy(nrm_sb, nrm)
                    nd = dsc.tile([NP], F32, name="nd", tag="nd")
                    nc.sync.dma_start(nd[:].rearrange("(a m) -> a m", a=1), nrm_sb)
                    nT = small.tile([128, NCHUNK], F32, name="nT", tag="nT")
                    nc.sync.dma_start(nT, nd[:].rearrange("(i p) -> p i", p=128))
                    rcp = small.tile([128, NCHUNK], F32, name="rcp", tag="rcp")
                    nc.vector.reciprocal(rcp, nT)
                    invT = small.tile([128, NCHUNK], F32, name="invT", tag="invT")
                    nc.scalar.activation(invT, rcp, AF.Sqrt)
                    nd2 = dsc.tile([NP], F32, name="nd2", tag="nd2")
                    nc.sync.dma_start(nd2[:].rearrange("(i p) -> p i", p=128), invT)
                    inv1 = small.tile([1, NP], F32, name="inv1", tag="inv1")
                    nc.sync.dma_start(inv1, nd2[:].rearrange("(a m) -> a m", a=1))
                    ibc = ps.tile([128, NP], F32, name="ibc", tag="bigbc")
                    for h in range(2):
                        nc.tensor.matmul(ibc[:, 512 * h:512 * (h + 1)], ones_r,
                                         inv1[:, 512 * h:512 * (h + 1)],
                                         start=True, stop=True)
                    z = big.tile([128, NP], F32R, name=f"z{b}{p}",
                                 tag=f"z{b}{p}", bufs=1)
                    nc.vector.tensor_tensor(
                        out=z.rearrange("c (a b) -> c a b", a=NPH), in0=zv,
                        in1=ibc.rearrange("c (a b) -> c a b", a=NPH), op=ALU.mult)
                    zb.append(z)
                zp.append(zb)

            # ---- phase M: per (batch, map q)
            for b in range(B_LOC):
                for q in range(3):
                    s_slab = slabp.tile([128, NCHUNK, NP], F32, name="s_slab",
                                        tag="s_slab")
                    rT = small.tile([128, NCHUNK], F32, name="rT", tag="rT")
                    srcs = [zp[b][0]] if q == 0 else \
                           [zp[b][1]] if q == 1 else [zp[b][0], zp[b][1]]
                    for i in range(NCHUNK):
                        G = ps.tile([128, NP], F32, name="G", tag="G")
                        for h in range(2):
                            for si, zs in enumerate(srcs):
                                nc.tensor.matmul(
                                    G[:, 512 * h:512 * (h + 1)],
                                    zs[:, 128 * i:128 * (i + 1)],
                                    zs[:, 512 * h:512 * (h + 1)],
                                    start=(si == 0), stop=(si == len(srcs) - 1))
                        nc.vector.scalar_tensor_tensor(
                            out=s_slab[:, i, :], in0=G, scalar=1.0,
                            in1=mask_sb[:, i, :], op0=ALU.mult, op1=ALU.mult)
                        e_scr = work.tile([128, NP], F32, name="e_scr", tag="e_scr")
                        nc.scalar.activation(e_scr, s_slab[:, i, :], AF.Exp,
                                             scale=scale_E[q],
                                             accum_out=rT[:, i:i + 1])
                    uT = small.tile([128, NCHUNK], F32, name="uT", tag="uT")
                    nc.vector.reciprocal(uT, rT)
                    u_dram = dsc.tile([NP], F32, name="u_dram", tag="u_dram")
                    nc.sync.dma_start(
                        u_dram[:].rearrange("(i p) -> p i", p=128), uT)
                    u_row = small.tile([1, NP], F32, name="u_row", tag="u_row")
                    nc.sync.dma_start(u_row, u_dram[:].rearrange("(a m) -> a m", a=1))
                    ubc = ps.tile([128, NP], F32, name="ubc", tag="bigbc")
                    for h in range(2):
                        nc.tensor.matmul(ubc[:, 512 * h:512 * (h + 1)], ones_r,
                                         u_row[:, 512 * h:512 * (h + 1)],
                                         start=True, stop=True)
                    ubc_sb = work.tile([128, NP], BF16, name="ubc_sb",
                                       tag="ubc_sb")
                    nc.scalar.copy(ubc_sb, ubc)
                    oacc = work.tile([128, NCHUNK, TOPK], F32, name="oacc",
                                     tag="oacc")
                    for i in range(NCHUNK):
                        f_scr = work.tile([128, NP], BF16, name="f_scr", tag="f_scr")
                        nc.scalar.activation(f_scr, s_slab[:, i, :], AF.Exp,
                                             scale=scale_F[q])
                        v_scr = work.tile([128, NP], BF16, name="v_scr", tag="v_scr")
                        nc.vector.tensor_tensor(out=v_scr, in0=f_scr, in1=ubc_sb,
                                                op=ALU.mult)
                        t8 = small.tile([128, 8], BF16, name="t8", tag="t8")
                        nc.vector.max(out=t8, in_=v_scr)
                        nc.vector.tensor_scalar_mul(
                            oacc[:, i, :], t8[:, :TOPK], uT[:, i:i + 1])
                    # out[b, q, 128i+p, k] <- oacc[p, i, k]
                    dst = out_d[b, q].rearrange("(i p) k -> p i k", p=128)
                    nc.sync.dma_start(dst, oacc)

    nc.compile()
    return nc


_NC_CACHE = None


def _get_nc():
    global _NC_CACHE
    if _NC_CACHE is None:
        _NC_CACHE = build_nc()
    return _NC_CACHE


def kernel(x: np.ndarray, alpha: np.ndarray) -> np.ndarray:
    global LAST_EXEC_NS
    x = np.ascontiguousarray(np.asarray(x, dtype=np.float32))
    alpha_arr = np.asarray(alpha, dtype=np.float32).reshape(1, 1)
    mask = _build_mask()

    nc = _get_nc()
    in_maps = []
    for core in range(N_CORES):
        xs = x[core * B_LOC:(core + 1) * B_LOC].reshape(B_LOC, C, H * W)
        in_maps.append({"x": np.ascontiguousarray(xs), "mask": mask,
                        "alpha": alpha_arr})
    res = run_bass_kernel_spmd(nc, in_maps, core_ids=list(range(N_CORES)))
    LAST_EXEC_NS = res.exec_time_ns

    # assemble: out[bg, k, 2i+dr, 2j+dc] from T_q[b, n=i*32+j, k]
    out = np.empty((B_FULL, TOPK, H, W), dtype=np.float32)
    for core in range(N_CORES):
        t = res.results[core]["out"]  # [B_LOC, 3, NP, TOPK]
        for bl in range(B_LOC):
            bg = core * B_LOC + bl
            tq = t[bl].reshape(3, NPH, NPH, TOPK).transpose(0, 3, 1, 2)
            out[bg, :, 0::2, 0::2] = tq[0]
            out[bg, :, 1::2, 1::2] = tq[1]
            out[bg, :, 0::2, 1::2] = tq[2]
            out[bg, :, 1::2, 0::2] = tq[2]
    return out


# revision 8
# speedup vs baseline: 1.2620x; 1.2620x over previous
"""Trainium2 Bass kernel for nn_LASCC (sparse patch-correlation attention + top-k).

Math (per batch element b):
  x_hat = L2-normalize(x, dim=channels)
  z_p[c, n] = x_hat at the two in-patch diagonal pixels (p=0: (0,0), p=1: (1,1))
  C_p = z_p^T z_p                  (1024x1024 normalized correlation, symmetric)
  C_2 = (C_0 + C_1)/2              (avg map)
  s_q = alpha * mask * C_q         (mask symmetric -> s symmetric)
  A_q = softmax_row(s) * softmax_col(s) = E^2 / (r[n] r[m]),  E=exp(s), r=rowsum(E)
  out pixel (row,col) with patch n, map q: top-3 over m of A_q[n, m]

Device kernel per core handles 2 batch elements (pure data parallel over b).
Top-3 of A[n,:] = (top-3 of F[n,:]*u[m]) * u[n]  with F = E^2 = exp(2*s), u = 1/r.
Top-8 per row via the DVE max8 instruction; first 3 taken.

The six (b, q) map-pipelines are software-pipelined at emission level:
E-phase (matmul + mask-mult + exp/rowsum) of stage k+1 is emitted before
F-phase (exp2 + u-scale + max8) of stage k, so each engine's in-order stream
has ready work while stage k's u row-sum reciprocal chain (DMA roundtrip)
completes.
"""
import numpy as np

import concourse.bass as bass
import concourse.mybir as mybir
from concourse import bacc
from concourse.tile import TileContext
from concourse.bass_utils import run_bass_kernel_spmd

F32 = mybir.dt.float32
F32R = mybir.dt.float32r
AF = mybir.ActivationFunctionType
ALU = mybir.AluOpType

B_FULL = 16
N_CORES = 8
B_LOC = B_FULL // N_CORES  # 2
C = 128
H = W = 64
NPH = 32
NP = 1024  # patches
PS = 2
TOPK = 3
NCHUNK = NP // 128  # 8

LAST_EXEC_NS = None


def _build_mask() -> np.ndarray:
    """(1 - gaussian) self-suppression mask, [NP, NP] float32 (matches reference)."""
    rat_s = np.float32(0.05)
    sr = np.float32(NPH) * rat_s
    ind_r = np.arange(NPH, dtype=np.float32).reshape(1, NPH, 1)
    ind_c = np.arange(NPH, dtype=np.float32).reshape(1, 1, NPH)
    cent = np.arange(NPH, dtype=np.float32)
    cent_r = np.repeat(cent, NPH).reshape(NP, 1, 1)
    cent_c = np.tile(cent, NPH).reshape(NP, 1, 1)
    g = np.exp(-((ind_r - cent_r) ** 2) / (2.0 * sr * sr)) * np.exp(
        -((ind_c - cent_c) ** 2) / (2.0 * sr * sr)
    )
    return (1.0 - g).reshape(NP, NP).astype(np.float32)


def build_nc():
    nc = bacc.Bacc(trn_type="TRN2")

    x_d = nc.dram_tensor("x", [B_LOC, C, H * W], F32, kind="ExternalInput")
    mask_d = nc.dram_tensor("mask", [NP, NP], F32, kind="ExternalInput")
    alpha_d = nc.dram_tensor("alpha", [1, 1], F32, kind="ExternalInput")
    out_d = nc.dram_tensor("out", [B_LOC, 3, NP, TOPK], F32, kind="ExternalOutput")

    with TileContext(nc) as tc:
        with tc.tile_pool(name="const", bufs=1) as cpool, \
             tc.tile_pool(name="big", bufs=1) as big, \
             tc.tile_pool(name="slab", bufs=2) as slabp, \
             tc.tile_pool(name="work", bufs=3) as work, \
             tc.tile_pool(name="small", bufs=3) as small, \
             tc.tile_pool(name="ps", bufs=2, space="PSUM") as ps, \
             tc.tile_pool(name="dsc", bufs=3, space="DRAM") as dsc:

            # ---- constants
            mask_sb = cpool.tile([128, NCHUNK, NP], F32)  # mask[128i+p, m] at [p,i,m]
            nc.sync.dma_start(
                mask_sb, mask_d[:, :].rearrange("(i p) m -> p i m", p=128)
            )
            ones_r = cpool.tile([1, 128], F32)  # K=1 bcast matmul lhsT
            nc.vector.memset(ones_r, 1.0)
            ones_k = cpool.tile([128, 1], F32)  # column-sum matmul lhsT
            nc.vector.memset(ones_k, 1.0)
            al_sb = cpool.tile([1, 1], F32)
            nc.sync.dma_start(al_sb, alpha_d[:, :])
            av = cpool.tile([128, 1], F32)  # alpha
            nc.gpsimd.partition_broadcast(av, al_sb)
            av_h = cpool.tile([128, 1], F32)  # alpha/2
            nc.vector.tensor_scalar_mul(av_h, av, 0.5)
            av_d = cpool.tile([128, 1], F32)  # 2*alpha
            nc.vector.tensor_scalar_mul(av_d, av, 2.0)
            scale_E = [av, av, av_h]
            scale_F = [av_d, av_d, av]

            # ---- phase N: norms + normalized z', stage-grouped across the
            # four independent (b, p) chains so they pipeline.
            chains = []  # (b, p, zv view)
            for b in range(B_LOC):
                xs = slabp.tile([128, H * W], F32, name=f"xs{b}", tag="s_slab")
                nc.sync.dma_start(xs, x_d[b])
                xr = xs.rearrange("c (i r j s) -> c r s i j", r=PS, s=PS, j=NPH)
                for p in range(PS):
                    chains.append((b, p, xr[:, p, p]))

            st = {}
            for b, p, zv in chains:  # stage 1: square + norm matmul + psum copy
                zsq = work.tile([128, NP], F32, name="zsq", tag="zsq", bufs=2)
                nc.scalar.activation(
                    zsq.rearrange("c (a b) -> c a b", a=NPH), zv, AF.Square)
                nrm = ps.tile([1, NP], F32, name="nrm", tag="bigbc")
                for h in range(2):
                    nc.tensor.matmul(nrm[:, 512 * h:512 * (h + 1)], ones_k,
                                     zsq[:, 512 * h:512 * (h + 1)],
                                     start=True, stop=True)
                nrm_sb = small.tile([1, NP], F32, name="nrm_sb", tag="nrm_sb", bufs=2)
                nc.scalar.copy(nrm_sb, nrm)
                st[(b, p)] = nrm_sb
            for b, p, zv in chains:  # stage 2: DRAM reshape to [128,8]
                nd = dsc.tile([NP], F32, name="nd", tag="nd", bufs=4)
                nc.sync.dma_start(nd[:].rearrange("(a m) -> a m", a=1), st[(b, p)])
                nT = small.tile([128, NCHUNK], F32, name="nT", tag="nT", bufs=4)
                nc.sync.dma_start(nT, nd[:].rearrange("(i p) -> p i", p=128))
                st[(b, p)] = nT
            for b, p, zv in chains:  # stage 3: rsqrt on [128,8]
                rcp = small.tile([128, NCHUNK], F32, name="rcp", tag="rcp", bufs=4)
                nc.vector.reciprocal(rcp, st[(b, p)])
                invT = small.tile([128, NCHUNK], F32, name="invT", tag="invT",
                                  bufs=4)
                nc.scalar.activation(invT, rcp, AF.Sqrt)
                st[(b, p)] = invT
            for b, p, zv in chains:  # stage 4: back to a [1, NP] row
                nd2 = dsc.tile([NP], F32, name="nd2", tag="nd2", bufs=4)
                nc.sync.dma_start(nd2[:].rearrange("(i p) -> p i", p=128),
                                  st[(b, p)])
                inv1 = small.tile([1, NP], F32, name="inv1", tag="inv1", bufs=2)
                nc.sync.dma_start(inv1, nd2[:].rearrange("(a m) -> a m", a=1))
                st[(b, p)] = inv1
            zp = {}
            for b, p, zv in chains:  # stage 5: broadcast + normalize z
                ibc = ps.tile([128, NP], F32, name="ibc", tag="bigbc")
                for h in range(2):
                    nc.tensor.matmul(ibc[:, 512 * h:512 * (h + 1)], ones_r,
                                     st[(b, p)][:, 512 * h:512 * (h + 1)],
                                     start=True, stop=True)
                z = big.tile([128, NP], F32R, name=f"z{b}{p}", tag=f"z{b}{p}",
                             bufs=1)
                nc.vector.tensor_tensor(
                    out=z.rearrange("c (a b) -> c a b", a=NPH), in0=zv,
                    in1=ibc.rearrange("c (a b) -> c a b", a=NPH), op=ALU.mult)
                zp[(b, p)] = z

            # ---- phase M: six (b, q) stages, software-pipelined.
            def emit_E(b, q):
                """matmuls + mask-mult + exp/rowsum; returns stage state."""
                s_slab = slabp.tile([128, NCHUNK, NP], F32, name="s_slab",
                                    tag="s_slab")
                rT = small.tile([128, NCHUNK], F32, name="rT", tag="rT")
                srcs = [zp[(b, 0)]] if q == 0 else \
                       [zp[(b, 1)]] if q == 1 else [zp[(b, 0)], zp[(b, 1)]]
                for i in range(NCHUNK):
                    G = ps.tile([128, NP], F32, name="G", tag="G")
                    for h in range(2):
                        for si, zs in enumerate(srcs):
                            nc.tensor.matmul(
                                G[:, 512 * h:512 * (h + 1)],
                                zs[:, 128 * i:128 * (i + 1)],
                                zs[:, 512 * h:512 * (h + 1)],
                                start=(si == 0), stop=(si == len(srcs) - 1))
                    nc.vector.scalar_tensor_tensor(
                        out=s_slab[:, i, :], in0=G, scalar=1.0,
                        in1=mask_sb[:, i, :], op0=ALU.mult, op1=ALU.mult)
                    e_scr = work.tile([128, NP], F32, name="e_scr", tag="e_scr", bufs=2)
                    nc.scalar.activation(e_scr, s_slab[:, i, :], AF.Exp,
                                         scale=scale_E[q],
                                         accum_out=rT[:, i:i + 1])
                uT = small.tile([128, NCHUNK], F32, name="uT", tag="uT")
                nc.vector.reciprocal(uT, rT)
                u_dram = dsc.tile([NP], F32, name="u_dram", tag="u_dram")
                nc.sync.dma_start(u_dram[:].rearrange("(i p) -> p i", p=128), uT)
                u_row = small.tile([1, NP], F32, name="u_row", tag="u_row", bufs=2)
                nc.sync.dma_start(u_row, u_dram[:].rearrange("(a m) -> a m", a=1))
                ubc = ps.tile([128, NP], F32, name="ubc", tag="bigbc")
                for h in range(2):
                    nc.tensor.matmul(ubc[:, 512 * h:512 * (h + 1)], ones_r,
                                     u_row[:, 512 * h:512 * (h + 1)],
                                     start=True, stop=True)
                return dict(s_slab=s_slab, uT=uT, ubc=ubc, b=b, q=q)

            def emit_F(stg):
                """second exp + u-scale + top-8 + rescale + store."""
                b, q = stg["b"], stg["q"]
                oacc = work.tile([128, NCHUNK, TOPK], F32, name="oacc", tag="oacc")
                for i in range(NCHUNK):
                    f_scr = work.tile([128, NP], F32, name="f_scr", tag="f_scr")
                    nc.scalar.activation(f_scr, stg["s_slab"][:, i, :], AF.Exp,
                                         scale=scale_F[q])
                    v_scr = work.tile([128, NP], F32, name="v_scr", tag="v_scr")
                    nc.vector.tensor_tensor(out=v_scr, in0=f_scr, in1=stg["ubc"],
                                            op=ALU.mult)
                    t8 = small.tile([128, 8], F32, name="t8", tag="t8")
                    nc.vector.max(out=t8, in_=v_scr)
                    nc.vector.tensor_scalar_mul(
                        oacc[:, i, :], t8[:, :TOPK], stg["uT"][:, i:i + 1])
                dst = out_d[b, q].rearrange("(i p) k -> p i k", p=128)
                nc.sync.dma_start(dst, oacc)

            stages = [(b, q) for b in range(B_LOC) for q in range(3)]
            pending = None
            for (b, q) in stages:
                stg = emit_E(b, q)
                if pending is not None:
                    emit_F(pending)
                pending = stg
            emit_F(pending)

    nc.compile()
    return nc


_NC_CACHE = None


def _get_nc():
    global _NC_CACHE
    if _NC_CACHE is None:
        _NC_CACHE = build_nc()
    return _NC_CACHE


def kernel(x: np.ndarray, alpha: np.ndarray) -> np.ndarray:
    global LAST_EXEC_NS
    x = np.ascontiguousarray(np.asarray(x, dtype=np.float32))
    alpha_arr = np.asarray(alpha, dtype=np.float32).reshape(1, 1)
    mask = _build_mask()

    nc = _get_nc()
    in_maps = []
    for core in range(N_CORES):
        xs = x[core * B_LOC:(core + 1) * B_LOC].reshape(B_LOC, C, H * W)
        in_maps.append({"x": np.ascontiguousarray(xs), "mask": mask,
                        "alpha": alpha_arr})
    res = run_bass_kernel_spmd(nc, in_maps, core_ids=list(range(N_CORES)))
    LAST_EXEC_NS = res.exec_time_ns

    # assemble: out[bg, k, 2i+dr, 2j+dc] from T_q[b, n=i*32+j, k]
    out = np.empty((B_FULL, TOPK, H, W), dtype=np.float32)
    for core in range(N_CORES):
        t = res.results[core]["out"]  # [B_LOC, 3, NP, TOPK]
        for bl in range(B_LOC):
            bg = core * B_LOC + bl
            tq = t[bl].reshape(3, NPH, NPH, TOPK).transpose(0, 3, 1, 2)
            out[bg, :, 0::2, 0::2] = tq[0]
            out[bg, :, 1::2, 1::2] = tq[1]
            out[bg, :, 0::2, 1::2] = tq[2]
            out[bg, :, 1::2, 0::2] = tq[2]
    return out
